# revision 1
# baseline (speedup 1.0000x reference)
"""Trainium2 Bass kernel for nn_Attention_83330955478086.

Full attention layer: QKV projections + (degenerate) rotary + causal softmax
attention + output projection.  x:(1,2048,4096), 32 heads x 128 head_dim.

Sharding: tensor-parallel over heads. Each of the 8 cores computes 4 heads
(d-shard of 512) of Q/K/V, runs attention for those heads, then the cores
AllGather the attention outputs (two 1024-seq chunks, pipelined against the
remaining attention work) and each computes a 512-column slice of the final
output projection.  Host concatenates the slices.

Layout: everything on-chip is "transposed" ([feature, seq]) so no on-device
transposes are needed anywhere:
  - host feeds x^T, wq^T, wk^T, wv^T, wo^T (marshalling)
  - Q/K projections emit Q^T/K^T tiles [head_dim, seq] directly
  - scores are computed transposed: scoresT[k,q] = sum_hd K^T[hd,k]*Q^T[hd,q]
  - softmax: exp on ACT; k-sums via ones-matmul; bcast-reciprocal normalize
  - PV uses V in natural [seq, d] layout as the stationary operand and emits
    attn^T [hd, q]; AllGather concatenates attn^T on the feature axis
  - output projection emits out^T [512, 2048]; host transposes back.

Rotary degenerates to an elementwise scale (the reference's pair-swap is the
identity): out[2j] = q[2j]*(c_j - s_j), out[2j+1] = q[2j+1]*(c_j + s_j).
We permute the wq/wk columns per head (even hd first, odd hd second, on the
host) so the device multiplies by a [cos-sin; cos+sin] stacked tile without
interleaved-partition access.  The permutation cancels in the q.k contraction.

Matmuls run as float32r (single-pass fp32, ~2e-4 rel err, full PE rate).
"""
import math
import os

import numpy as np

import concourse.bacc as bacc
import concourse.tile as tile
from concourse.tile import add_dep_helper
from concourse import mybir
from concourse.bass_utils import run_bass_kernel_spmd

N_CORES = 8
S = 2048
D = 4096
H = 32
HD = 128
DSH = D // N_CORES  # 512 per-core d shard
HL = DSH // HD  # 4 heads per core
KT = D // 128  # 32 contraction tiles for the projections
SC = S // 512  # 4 seq chunks of 512
ST = S // 128  # 16 seq tiles of 128

F32 = mybir.dt.float32
F32R = mybir.dt.float32r

# mask-block classes
B_SKIP = 0  # fully masked (mask < -1e4): exp underflows to exactly 0 -> skip
B_ZERO = 1  # mask identically 0: skip the add
B_ADD = 2  # mixed: stream the mask tile and add


def _w_load(nc, sb_tile, dram, kt0, kt1, n_chunks, engines):
    """Load kc tiles [kt0, kt1) of a [D, DSH] weight into `sb_tile` (kc-major
    [128, (kt1-kt0)*DSH]) in chunks so consumers start after ~1/n of the load."""
    ktn = kt1 - kt0
    kt_per = ktn // n_chunks
    for g in range(n_chunks):
        eng = getattr(nc, engines[g % len(engines)])
        eng.dma_start(
            sb_tile[:, g * kt_per * DSH : (g + 1) * kt_per * DSH].rearrange(
                "p (t d) -> p t d", d=DSH
            ),
            dram.ap()[
                (kt0 + g * kt_per) * 128 : (kt0 + (g + 1) * kt_per) * 128, :
            ].rearrange("(t p) d -> p t d", p=128),
        )


def _build(cls_grid):
    nc = bacc.Bacc(
        "TRN2", target_bir_lowering=False, debug=False, num_devices=N_CORES
    )

    xT = nc.dram_tensor("xT", [D, S], F32R, kind="ExternalInput")
    wqT = nc.dram_tensor("wqT", [D, DSH], F32R, kind="ExternalInput")
    wkT = nc.dram_tensor("wkT", [D, DSH], F32R, kind="ExternalInput")
    wvT = nc.dram_tensor("wvT", [D, DSH], F32R, kind="ExternalInput")
    woT = nc.dram_tensor("woT", [D, DSH], F32R, kind="ExternalInput")
    freqT = nc.dram_tensor("freqT", [128, S], F32, kind="ExternalInput")
    ones_in = nc.dram_tensor("ones_in", [128, 1], F32R, kind="ExternalInput")
    maskT = nc.dram_tensor("maskT", [S, S], F32, kind="ExternalInput")
    outT = nc.dram_tensor("outT", [DSH, S], F32, kind="ExternalOutput")

    qT_d = nc.dram_tensor("qT_d", [DSH, S], F32R)
    kT_d = nc.dram_tensor("kT_d", [DSH, S], F32R)
    attn_sc = [nc.dram_tensor(f"attn_sc{i}", [DSH, 1024], F32R) for i in range(2)]
    attn_full = [
        nc.dram_tensor(f"attn_full{i}", [D, 1024], F32R, addr_space="Shared")
        for i in range(2)
    ]

    with tile.TileContext(nc) as tc, tc.tile_pool(
        name="pv_keep", bufs=1
    ) as pv_keep, tc.tile_pool(name="p2_k", bufs=1) as p2_k:
        v_sb = pv_keep.tile([128, ST * DSH], F32R)  # V, persists to phase 2
        kres = [
            p2_k.tile([128, S], F32R, name=f"kres{h}") for h in range(HL)
        ]

        # ---- passes A1/A2/B: Q^T, K^T (transposed) and V projections ----
        with (
            tc.tile_pool(name="px", bufs=6) as px,
            tc.tile_pool(name="pg", bufs=1) as pg,
            tc.tile_pool(name="pev", bufs=4) as pev,
        ):
            # rotary multipliers: gk rows 0-63 = cos-sin, 64-127 = cos+sin;
            # gq = gk/sqrt(HD).  DVE tensor_tensor needs equal base
            # partitions, so compute in [64, *] tiles, assemble via DMA.
            gk = pg.tile([128, S], F32)
            gq = pg.tile([128, S], F32)
            with tc.tile_pool(name="pa_f", bufs=1) as pa_f:
                HS = S // 2
                for fh in range(2):
                    fcos = pa_f.tile([64, HS], F32, name="fcos")
                    nc.sync.dma_start(
                        fcos[:], freqT.ap()[0:64, fh * HS : (fh + 1) * HS]
                    )
                    fsin = pa_f.tile([64, HS], F32, name="fsin")
                    nc.sync.dma_start(
                        fsin[:], freqT.ap()[64:128, fh * HS : (fh + 1) * HS]
                    )
                    cms = pa_f.tile([64, HS], F32, name="cms")
                    nc.vector.tensor_sub(cms[:], fcos[:], fsin[:])
                    cps = pa_f.tile([64, HS], F32, name="cps")
                    nc.vector.tensor_add(cps[:], fcos[:], fsin[:])
                    nc.sync.dma_start(gk[0:64, fh * HS : (fh + 1) * HS], cms[:])
                    nc.sync.dma_start(
                        gk[64:128, fh * HS : (fh + 1) * HS], cps[:]
                    )
                nc.scalar.mul(gq[:], gk[:], 1.0 / math.sqrt(HD))

            def proj_qk(w_dram, out_dram, g_tile, ev_name, pw):
                """Transposed projection: out[d, s] = w^T.T @ x^T with the
                rotary multiplier applied on evacuation."""
                w_sb = pw.tile([128, KT * DSH], F32R, name="w_sb")
                _w_load(nc, w_sb, w_dram, 0, KT, 8, ("sync", "scalar"))
                with tc.tile_pool(name="qk_ps", bufs=1, space="PSUM") as ps:
                    for sp in range(2):  # seq-chunk pairs
                        psd = [
                            ps.tile([128, 512], F32, name=f"ps{i}")
                            for i in range(8)
                        ]
                        for kc in range(KT):
                            xt = px.tile([128, 1024], F32R, name="xs")
                            eng = nc.sync if kc % 2 == 0 else nc.scalar
                            eng.dma_start(
                                xt[:],
                                xT.ap()[
                                    kc * 128 : (kc + 1) * 128,
                                    sp * 1024 : (sp + 1) * 1024,
                                ],
                            )
                            for half in range(2):
                                for dt in range(HL):
                                    nc.tensor.matmul(
                                        psd[half * HL + dt][:],
                                        w_sb[
                                            :,
                                            kc * DSH
                                            + dt * 128 : kc * DSH
                                            + (dt + 1) * 128,
                                        ],
                                        xt[:, half * 512 : (half + 1) * 512],
                                        start=(kc == 0),
                                        stop=(kc == KT - 1),
                                    )
                        for half in range(2):
                            sc = sp * 2 + half
                            for dt in range(HL):
                                ev = pev.tile(
                                    [128, 512], F32R, name=ev_name
                                )
                                nc.vector.tensor_mul(
                                    ev[:],
                                    psd[half * HL + dt][:],
                                    g_tile[:, sc * 512 : (sc + 1) * 512],
                                )
                                eng = nc.sync if dt % 2 == 0 else nc.scalar
                                eng.dma_start(
                                    out_dram.ap()[
                                        dt * 128 : (dt + 1) * 128,
                                        sc * 512 : (sc + 1) * 512,
                                    ],
                                    ev[:],
                                )

            with tc.tile_pool(name="pw_q", bufs=1) as pw_q:
                proj_qk(wqT, qT_d, gq, "qev", pw_q)
            with tc.tile_pool(name="pw_k", bufs=1) as pw_k:
                proj_qk(wkT, kT_d, gk, "kev", pw_k)

            # K prefetch for phase 2 (kT_d complete now)
            for h in range(HL):
                nc.gpsimd.dma_start(
                    kres[h][:], kT_d.ap()[h * 128 : (h + 1) * 128, :]
                )

            # pass B: V in natural [s, d] layout (x tiles are stationary)
            with (
                tc.tile_pool(name="pw_v", bufs=1) as pw_v,
                tc.tile_pool(name="pb_ps", bufs=1, space="PSUM") as pb_ps,
            ):
                wv_sb = pw_v.tile([128, KT * DSH], F32R)
                _w_load(nc, wv_sb, wvT, 0, KT, 8, ("gpsimd",))
                for sh in range(2):
                    psv = [
                        pb_ps.tile([128, 512], F32, name=f"psv{i}")
                        for i in range(8)
                    ]
                    for kc in range(KT):
                        xt2 = px.tile([128, 1024], F32R, name="xs")
                        eng = nc.sync if kc % 2 == 0 else nc.scalar
                        eng.dma_start(
                            xt2[:],
                            xT.ap()[
                                kc * 128 : (kc + 1) * 128,
                                sh * 1024 : (sh + 1) * 1024,
                            ],
                        )
                        for st in range(8):
                            nc.tensor.matmul(
                                psv[st][:],
                                xt2[:, st * 128 : (st + 1) * 128],
                                wv_sb[:, kc * DSH : (kc + 1) * DSH],
                                start=(kc == 0),
                                stop=(kc == KT - 1),
                            )
                    for st in range(8):
                        gt = sh * 8 + st  # global s-tile 0..15
                        nc.vector.tensor_copy(
                            v_sb[:, gt * DSH : (gt + 1) * DSH], psv[st][:]
                        )

        # ------ phase 2+3: attention, AllGather, output projection ------
        with (
            tc.tile_pool(name="p2_q", bufs=3) as p2_q,
            tc.tile_pool(name="p2_m", bufs=2) as p2_m,
            tc.tile_pool(name="p2_ex", bufs=6) as p2_ex,
            tc.tile_pool(name="p2_sm", bufs=3) as p2_sm,
            tc.tile_pool(name="p2_at", bufs=3) as p2_at,
            tc.tile_pool(name="p3_w", bufs=1) as p3_w,
            tc.tile_pool(name="p3_a", bufs=4) as p3_a,
            tc.tile_pool(name="p3_ev", bufs=4) as p3_ev,
            tc.tile_pool(name="p2_one", bufs=1) as p2_one,
        ):
            ones_t = p2_one.tile([128, 1], F32R)
            nc.sync.dma_start(ones_t[:], ones_in.ap())
            wo_sb = p3_w.tile([128, KT * DSH], F32R)
            _w_load(nc, wo_sb, woT, 0, KT, 8, ("gpsimd",))

            last_attn = {}

            def do_attn(qc, pool_sc, pool_ap, pool_sp):
                live = [kt for kt in range(ST) if cls_grid[kt][qc] != B_SKIP]
                mtiles = {}
                for kt in live:
                    if cls_grid[kt][qc] == B_ADD:
                        mk = p2_m.tile([128, 512], F32, name=f"mk{kt % 4}")
                        nc.scalar.dma_start(
                            mk[:],
                            maskT.ap()[
                                kt * 128 : (kt + 1) * 128,
                                qc * 512 : (qc + 1) * 512,
                            ],
                        )
                        mtiles[kt] = mk
                for h in range(HL):
                    qt = p2_q.tile([128, 512], F32R, name="qt")
                    nc.sync.dma_start(
                        qt[:],
                        qT_d.ap()[
                            h * 128 : (h + 1) * 128, qc * 512 : (qc + 1) * 512
                        ],
                    )
                    att_ps = pool_ap.tile([128, 512], F32, name="att_ps")
                    sum_ps = pool_sp.tile([1, 512], F32, name="sum_ps")
                    for i, kt in enumerate(live):
                        first = i == 0
                        last = i == len(live) - 1
                        sc_ps = pool_sc.tile([128, 512], F32, name="sc_ps")
                        nc.tensor.matmul(
                            sc_ps[:],
                            kres[h][:, kt * 128 : (kt + 1) * 128],
                            qt[:],
                            start=True,
                            stop=True,
                        )
                        if cls_grid[kt][qc] == B_ADD:
                            nc.vector.tensor_add(
                                sc_ps[:], sc_ps[:], mtiles[kt][:]
                            )
                        ex = p2_ex.tile([128, 512], F32R, name="ex")
                        last_attn["scalar"] = nc.scalar.activation(
                            ex[:], sc_ps[:], mybir.ActivationFunctionType.Exp
                        )
                        nc.tensor.matmul(
                            att_ps[:],
                            v_sb[
                                :, kt * DSH + h * 128 : kt * DSH + (h + 1) * 128
                            ],
                            ex[:],
                            start=first,
                            stop=last,
                        )
                        nc.tensor.matmul(
                            sum_ps[:],
                            ones_t[:],
                            ex[:],
                            start=first,
                            stop=last,
                        )
                    rec = p2_sm.tile([1, 512], F32, name="rec")
                    nc.vector.reciprocal_approx_fast(rec[:], sum_ps[0:1, :])
                    rb = p2_sm.tile([128, 512], F32, name="rb")
                    nc.gpsimd.partition_broadcast(rb[:], rec[0:1, :])
                    at = p2_at.tile([128, 512], F32R, name="at")
                    nc.vector.tensor_mul(at[:], att_ps[:], rb[:])
                    last_attn["sync"] = nc.sync.dma_start(
                        attn_sc[qc // 2].ap()[
                            h * 128 : (h + 1) * 128,
                            (qc % 2) * 512 : (qc % 2) * 512 + 512,
                        ],
                        at[:],
                    )

                if qc % 2 == 1:
                    nc.gpsimd.collective_compute(
                        "AllGather",
                        mybir.AluOpType.bypass,
                        ins=[attn_sc[qc // 2].ap()],
                        outs=[attn_full[qc // 2].ap()],
                        replica_groups=[list(range(N_CORES))],
                    )

            def do_p3(qc, pool_ps):
                pso = [
                    pool_ps.tile([128, 512], F32, name=f"pso{i}")
                    for i in range(HL)
                ]
                for dc2 in range(KT // 2):
                    at_t = p3_a.tile([128, 1024], F32R, name="at_t")
                    ename = "sync" if dc2 % 2 == 0 else "scalar"
                    eng = getattr(nc, ename)
                    ld = eng.dma_start(
                        at_t[:].rearrange("p (two s) -> p two s", s=512),
                        attn_full[qc // 2]
                        .ap()[
                            dc2 * 256 : (dc2 + 1) * 256,
                            (qc % 2) * 512 : (qc % 2) * 512 + 512,
                        ]
                        .rearrange("(two p) s -> p two s", p=128),
                    )
                    if dc2 < 2 and ename in last_attn:
                        add_dep_helper(
                            ld.ins,
                            last_attn[ename].ins,
                            sync=False,
                            reason="p3 loads after attention DMAs",
                        )
                    for half in range(2):
                        dc = dc2 * 2 + half
                        for jt in range(HL):
                            nc.tensor.matmul(
                                pso[jt][:],
                                wo_sb[
                                    :,
                                    dc * DSH
                                    + jt * 128 : dc * DSH
                                    + (jt + 1) * 128,
                                ],
                                at_t[:, half * 512 : (half + 1) * 512],
                                start=(dc == 0),
                                stop=(dc == KT - 1),
                            )
                for jt in range(HL):
                    oev = p3_ev.tile([128, 512], F32, name="oev")
                    nc.vector.tensor_copy(oev[:], pso[jt][:])
                    nc.sync.dma_start(
                        outT.ap()[
                            jt * 128 : (jt + 1) * 128, qc * 512 : (qc + 1) * 512
                        ],
                        oev[:],
                    )

            # attention first (deep PE lookahead via 4 score banks), then
            # the output-projection chunks; the AllGathers fly while the PE
            # is still busy with later attention chunks.
            with (
                tc.tile_pool(name="p2_sc", bufs=4, space="PSUM") as psc,
                tc.tile_pool(name="p2_ap", bufs=2, space="PSUM") as pap,
                tc.tile_pool(name="p2_sp", bufs=2, space="PSUM") as psp,
            ):
                for qc in range(SC):
                    do_attn(qc, psc, pap, psp)
            with tc.tile_pool(name="p3_ps", bufs=1, space="PSUM") as pps:
                for qc in range(SC):
                    do_p3(qc, pps)

    nc.compile()
    return nc


def _install_trace_hooks():
    """Install the NTFF profile hook (missing antenv.axon_hooks stub) and
    neutralize the artifact upload so trace=True works in this container."""
    import sys
    import types

    from concourse import bass_utils as _bu

    _bu.upload_artifacts = lambda tmpdir: f"file://{tmpdir}"
    if "antenv.axon_hooks" in sys.modules:
        return
    import antenv

    mod = types.ModuleType("antenv.axon_hooks")
    _h = [None]
    mod.set_axon_ntff_profile_hook = lambda hk: _h.__setitem__(0, hk)
    mod.get_axon_ntff_profile_hook = lambda: _h[0]
    sys.modules["antenv.axon_hooks"] = mod
    antenv.axon_hooks = mod
    from trn_agent_boot.trn_boot import _ntff_profile_via_ctypes

    mod.set_axon_ntff_profile_hook(
        _ntff_profile_via_ctypes("/opt/axon/libaxon_pjrt.so")
    )


_CACHE = {}


def _get_program(cls_grid):
    key = tuple(map(tuple, cls_grid))
    if key not in _CACHE:
        _CACHE[key] = _build(cls_grid)
    return _CACHE[key]


def _classify_mask(maskT_np):
    """Classify each [128k, 512q] block of the transposed mask."""
    grid = []
    for kt in range(ST):
        row = []
        for qc in range(SC):
            blk = maskT_np[kt * 128 : (kt + 1) * 128, qc * 512 : (qc + 1) * 512]
            if np.all(blk < -1e4):
                row.append(B_SKIP)
            elif np.all(blk == 0.0):
                row.append(B_ZERO)
            else:
                row.append(B_ADD)
        grid.append(row)
    return grid


_ONES = np.ones((128, 1), dtype=np.float32)

# within-head permutation: even head_dim indices first, then odd
_PERM = np.empty(DSH, dtype=np.int64)
for _hl in range(HL):
    for _j in range(64):
        _PERM[_hl * 128 + _j] = _hl * 128 + 2 * _j
        _PERM[_hl * 128 + 64 + _j] = _hl * 128 + 2 * _j + 1


def kernel(x, start_pos, freqs, mask, wq, wk, wv, wo):
    x = np.asarray(x, dtype=np.float32)
    freqs = np.asarray(freqs, dtype=np.float32)
    mask = np.asarray(mask, dtype=np.float32)
    wq = np.asarray(wq, dtype=np.float32)
    wk = np.asarray(wk, dtype=np.float32)
    wv = np.asarray(wv, dtype=np.float32)
    wo = np.asarray(wo, dtype=np.float32)

    xs = x.reshape(S, D)
    xT = np.ascontiguousarray(xs.T)
    freqT = np.ascontiguousarray(
        np.concatenate([freqs[:, :, 0].T, freqs[:, :, 1].T], axis=0)
    )  # [128, S]: rows 0-63 cos_j(s), 64-127 sin_j(s)
    maskT_np = np.ascontiguousarray(mask.reshape(S, S).T)
    cls_grid = _classify_mask(maskT_np)
    nc = _get_program(cls_grid)

    in_maps = []
    for c in range(N_CORES):
        rows = slice(c * DSH, (c + 1) * DSH)
        wq_c = wq[rows][_PERM]  # permute within-head rows (even hd, odd hd)
        wk_c = wk[rows][_PERM]
        in_maps.append(
            {
                "xT": xT,
                "wqT": np.ascontiguousarray(wq_c.T),
                "wkT": np.ascontiguousarray(wk_c.T),
                "wvT": np.ascontiguousarray(wv[rows].T),
                "woT": np.ascontiguousarray(wo[rows].T),
                "freqT": freqT,
                "ones_in": _ONES,
                "maskT": maskT_np,
            }
        )

    trace = os.environ.get("ATTN_TRACE") == "1"
    if trace:
        try:
            _install_trace_hooks()
        except Exception:
            pass

    res = run_bass_kernel_spmd(
        nc,
        in_maps,
        list(range(N_CORES)),
        trace=trace,
        trace_cores=[0] if trace else None,
    )
    if trace:
        kernel.last_exec_time_ns = res.exec_time_ns
        kernel.last_results = res

    out = np.empty((S, D), dtype=np.float32)
    for c in range(N_CORES):
        out[:, c * DSH : (c + 1) * DSH] = res.results[c]["outT"].T
    return out[None]



# revision 2
# speedup vs baseline: 1.1885x; 1.1885x over previous
"""Trainium2 Bass kernel for nn_Attention_83330955478086.

Full attention layer: QKV projections + (degenerate) rotary + causal softmax
attention + output projection.  x:(1,2048,4096), 32 heads x 128 head_dim.

Sharding: tensor-parallel over heads. Each of the 8 cores computes 4 heads
(d-shard of 512) of Q/K/V, runs attention for those heads, AllGathers the
attention outputs (two 1024-seq chunks) and computes a 512-column slice of
the final output projection.  Host concatenates the slices.

v2 design (vs the fp32r baseline):
  - everything on the wire is bf16 (x, weights, q/k/v, exp tiles, the
    attention exchange, the output); PSUM accumulation stays fp32.
  - Q^T/K^T/V projections evacuate straight into SBUF-resident tiles
    (qres/kres/v_sb) -- no DRAM roundtrips.
  - rotary multipliers (cos-sin / cos+sin stacks) and the diagonal causal
    mask patterns are precomputed on the host and streamed once.
  - weight tiles stream on the otherwise-idle Pool queue, double-buffered
    so each pass's weights load during the previous pass.
  - attention processes heads in pairs: two score matmuls fill one
    [128,1024] PSUM pair, one ACT exp covers both heads, softmax sums via
    ones-matmuls, normalization off the critical path via fp32 SBUF copies.
  - matmul loops ordered so consecutive matmuls share a stationary operand
    (halves LDWEIGHTS pressure in Q/K/p3).

Layout: everything on-chip is "transposed" ([feature, seq]) so no on-device
transposes are needed anywhere (scores computed as scoresT[k,q]).
"""
import math
import os

import numpy as np
import ml_dtypes

import concourse.bacc as bacc
import concourse.tile as tile
from concourse import mybir
from concourse.bass_utils import run_bass_kernel_spmd

N_CORES = 8
S = 2048
D = 4096
H = 32
HD = 128
DSH = D // N_CORES  # 512 per-core d shard
HL = DSH // HD  # 4 heads per core
KT = D // 128  # 32 contraction tiles for the projections
SC = S // 512  # 4 seq chunks of 512
ST = S // 128  # 16 seq tiles of 128

F32 = mybir.dt.float32
BF16 = mybir.dt.bfloat16

# mask-block classes
B_SKIP = -2  # fully masked: exp underflows to exactly 0 -> skip block
B_ZERO = -1  # mask identically 0: no add needed
# >= 0: index into the streamed mask patterns (additive [128, 1024] tiles)


def _w_load(nc, sb_tile, dram, n_chunks, engine):
    """Load a [D, DSH] weight into `sb_tile` (kc-major [128, KT*DSH]) in
    chunks so consumers can start after ~1/n of the load."""
    kt_per = KT // n_chunks
    for g in range(n_chunks):
        engine.dma_start(
            sb_tile[:, g * kt_per * DSH : (g + 1) * kt_per * DSH].rearrange(
                "p (t d) -> p t d", d=DSH
            ),
            dram.ap()[g * kt_per * 128 : (g + 1) * kt_per * 128, :].rearrange(
                "(t p) d -> p t d", p=128
            ),
        )


def _build(cls_grid, n_patterns):
    nc = bacc.Bacc(
        "TRN2", target_bir_lowering=False, debug=False, num_devices=N_CORES
    )

    xT = nc.dram_tensor("xT", [D, S], BF16, kind="ExternalInput")
    wqT = nc.dram_tensor("wqT", [D, DSH], BF16, kind="ExternalInput")
    wkT = nc.dram_tensor("wkT", [D, DSH], BF16, kind="ExternalInput")
    wvT = nc.dram_tensor("wvT", [D, DSH], BF16, kind="ExternalInput")
    woT = nc.dram_tensor("woT", [D, DSH], BF16, kind="ExternalInput")
    gq_d = nc.dram_tensor("gq", [128, S], F32, kind="ExternalInput")
    gk_d = nc.dram_tensor("gk", [128, S], F32, kind="ExternalInput")
    ones_in = nc.dram_tensor("ones_in", [128, 1], BF16, kind="ExternalInput")
    maskp_d = nc.dram_tensor(
        "maskp", [128, max(n_patterns, 1) * 1024], BF16, kind="ExternalInput"
    )
    outT = nc.dram_tensor("outT", [DSH, S], BF16, kind="ExternalOutput")

    attn_sc = [nc.dram_tensor(f"attn_sc{i}", [DSH, 1024], BF16) for i in range(2)]
    attn_full = [
        nc.dram_tensor(f"attn_full{i}", [D, 1024], BF16, addr_space="Shared")
        for i in range(2)
    ]

    with tile.TileContext(nc) as tc, tc.tile_pool(
        name="keep", bufs=1
    ) as keep, tc.tile_pool(name="pw", bufs=2) as pw:
        # ---- persistent SBUF tiles ----
        v_sb = keep.tile([128, ST * DSH], BF16)  # V  [s-tile-major, d]
        kres = [keep.tile([128, S], BF16, name=f"kres{h}") for h in range(HL)]
        qres = [keep.tile([128, S], BF16, name=f"qres{h}") for h in range(HL)]
        gq = keep.tile([128, S], F32)
        gk = keep.tile([128, S], F32)
        maskp = keep.tile([128, max(n_patterns, 1) * 1024], BF16)
        ones_t = keep.tile([128, 1], BF16)
        junk = keep.tile([128, 8], F32)

        nc.scalar.dma_start(gq[:], gq_d.ap())
        nc.scalar.dma_start(gk[:], gk_d.ap())
        nc.scalar.dma_start(maskp[:], maskp_d.ap())
        nc.sync.dma_start(ones_t[:], ones_in.ap())
        # warm the ACT exp table set before attention needs it
        nc.scalar.activation(
            junk[:, 0:1], ones_t[:], mybir.ActivationFunctionType.Exp
        )

        # weights stream on the Pool queue, double buffered: wk loads during
        # the Q pass, wv during K, wo during V/attention.
        wq_sb = pw.tile([128, KT * DSH], BF16, name="w")
        _w_load(nc, wq_sb, wqT, 8, nc.gpsimd)
        wk_sb = pw.tile([128, KT * DSH], BF16, name="w")
        _w_load(nc, wk_sb, wkT, 8, nc.gpsimd)

        with tc.tile_pool(name="px", bufs=8) as px, tc.tile_pool(
            name="proj_ps", bufs=1, space="PSUM"
        ) as pps:

            def xt_load(kc, sp):
                xt = px.tile([128, 1024], BF16, name="xs")
                eng = nc.sync if kc % 2 == 0 else nc.scalar
                eng.dma_start(
                    xt[:],
                    xT.ap()[
                        kc * 128 : (kc + 1) * 128, sp * 1024 : (sp + 1) * 1024
                    ],
                )
                return xt

            def proj_qk(w_sb, g_tile, res):
                """Transposed projection with rotary applied on evacuation:
                res[dt][hd, s] for the 4 head-tiles of this d-shard."""
                for sp in range(2):
                    psd = [
                        pps.tile([128, 512], F32, name=f"ps{i}") for i in range(8)
                    ]
                    for kc in range(KT):
                        xt = xt_load(kc, sp)
                        for dt in range(HL):
                            for half in range(2):  # same stationary back2back
                                nc.tensor.matmul(
                                    psd[half * HL + dt][:],
                                    w_sb[
                                        :,
                                        kc * DSH + dt * 128 : kc * DSH
                                        + (dt + 1) * 128,
                                    ],
                                    xt[:, half * 512 : (half + 1) * 512],
                                    start=(kc == 0),
                                    stop=(kc == KT - 1),
                                )
                    for half in range(2):
                        sc = sp * 2 + half
                        for dt in range(HL):
                            nc.vector.tensor_mul(
                                res[dt][:, sc * 512 : (sc + 1) * 512],
                                psd[half * HL + dt][:],
                                g_tile[:, sc * 512 : (sc + 1) * 512],
                            )

            proj_qk(wq_sb, gq, qres)
            proj_qk(wk_sb, gk, kres)

            wv_sb = pw.tile([128, KT * DSH], BF16, name="w")
            _w_load(nc, wv_sb, wvT, 8, nc.gpsimd)

            # V in natural [s, d] layout (x tiles are stationary)
            for sh in range(2):
                psv = [pps.tile([128, 512], F32, name=f"ps{i}") for i in range(8)]
                for kc in range(KT):
                    xt = xt_load(kc, sh)
                    for st in range(8):
                        nc.tensor.matmul(
                            psv[st][:],
                            xt[:, st * 128 : (st + 1) * 128],
                            wv_sb[:, kc * DSH : (kc + 1) * DSH],
                            start=(kc == 0),
                            stop=(kc == KT - 1),
                        )
                for st in range(8):
                    gt = sh * 8 + st
                    nc.vector.tensor_copy(
                        v_sb[:, gt * DSH : (gt + 1) * DSH], psv[st][:]
                    )

        wo_sb = pw.tile([128, KT * DSH], BF16, name="w")
        _w_load(nc, wo_sb, woT, 8, nc.gpsimd)

        # ------ attention + AllGather ------
        with (
            tc.tile_pool(name="p2_ex", bufs=4) as p2_ex,
            tc.tile_pool(name="p2_ac", bufs=4) as p2_ac,
            tc.tile_pool(name="p2_sm", bufs=2) as p2_sm,
            tc.tile_pool(name="p2_at", bufs=4) as p2_at,
            tc.tile_pool(name="p2_sc", bufs=2, space="PSUM") as psc,
            tc.tile_pool(name="p2_ap", bufs=1, space="PSUM") as pap,
            tc.tile_pool(name="p2_sp", bufs=1, space="PSUM") as psp,
        ):
            for qc in range(SC):
                live = [kt for kt in range(ST) if cls_grid[kt][qc] != B_SKIP]
                for hp in range(HL // 2):
                    heads = (2 * hp, 2 * hp + 1)
                    att = [
                        pap.tile([128, 512], F32, name=f"att{j}") for j in range(2)
                    ]
                    sum_ps = [
                        psp.tile([1, 512], F32, name=f"sum{j}") for j in range(2)
                    ]
                    for i, kt in enumerate(live):
                        first, last = i == 0, i == len(live) - 1
                        sc_ps = psc.tile([128, 1024], F32, name="sc")
                        for j, h in enumerate(heads):
                            nc.tensor.matmul(
                                sc_ps[:, j * 512 : (j + 1) * 512],
                                kres[h][:, kt * 128 : (kt + 1) * 128],
                                qres[h][:, qc * 512 : (qc + 1) * 512],
                                start=True,
                                stop=True,
                            )
                        pi = cls_grid[kt][qc]
                        if pi >= 0:
                            nc.vector.tensor_add(
                                sc_ps[:],
                                sc_ps[:],
                                maskp[:, pi * 1024 : (pi + 1) * 1024],
                            )
                        ex = p2_ex.tile([128, 1024], BF16, name="ex")
                        nc.scalar.activation(
                            ex[:], sc_ps[:], mybir.ActivationFunctionType.Exp
                        )
                        for j, h in enumerate(heads):
                            nc.tensor.matmul(
                                att[j][:],
                                v_sb[
                                    :,
                                    kt * DSH + h * 128 : kt * DSH + (h + 1) * 128,
                                ],
                                ex[:, j * 512 : (j + 1) * 512],
                                start=first,
                                stop=last,
                            )
                            nc.tensor.matmul(
                                sum_ps[j][:],
                                ones_t[:],
                                ex[:, j * 512 : (j + 1) * 512],
                                start=first,
                                stop=last,
                            )
                    # drain: copy att to SBUF fp32 (frees the PSUM banks),
                    # then normalize off the critical path.
                    attc = [
                        p2_ac.tile([128, 512], F32, name=f"attc{j}")
                        for j in range(2)
                    ]
                    rec = p2_sm.tile([1, 1024], F32, name="rec")
                    for j in range(2):
                        nc.vector.tensor_copy(attc[j][:], att[j][:])
                        nc.vector.reciprocal_approx_fast(
                            rec[0:1, j * 512 : (j + 1) * 512], sum_ps[j][0:1, :]
                        )
                    rb = p2_sm.tile([128, 1024], F32, name="rb")
                    nc.gpsimd.partition_broadcast(rb[:], rec[0:1, :])
                    for j, h in enumerate(heads):
                        at = p2_at.tile([128, 512], BF16, name="at")
                        nc.vector.tensor_mul(
                            at[:], attc[j][:], rb[:, j * 512 : (j + 1) * 512]
                        )
                        nc.sync.dma_start(
                            attn_sc[qc // 2].ap()[
                                h * 128 : (h + 1) * 128,
                                (qc % 2) * 512 : (qc % 2) * 512 + 512,
                            ],
                            at[:],
                        )
                if qc % 2 == 1:
                    nc.gpsimd.collective_compute(
                        "AllGather",
                        mybir.AluOpType.bypass,
                        ins=[attn_sc[qc // 2].ap()],
                        outs=[attn_full[qc // 2].ap()],
                        replica_groups=[list(range(N_CORES))],
                    )

        # ------ output projection ------
        with (
            tc.tile_pool(name="p3_a", bufs=8) as p3_a,
            tc.tile_pool(name="p3_ev", bufs=4) as p3_ev,
            tc.tile_pool(name="p3_ps", bufs=1, space="PSUM") as pps3,
        ):
            for qcp in range(2):
                pso = [
                    pps3.tile([128, 512], F32, name=f"pso{i}") for i in range(8)
                ]
                for dc in range(KT):
                    at_t = p3_a.tile([128, 1024], BF16, name="at_t")
                    eng = nc.sync if dc % 2 == 0 else nc.scalar
                    eng.dma_start(
                        at_t[:], attn_full[qcp].ap()[dc * 128 : (dc + 1) * 128, :]
                    )
                    for jt in range(HL):
                        for qch in range(2):  # same stationary back-to-back
                            nc.tensor.matmul(
                                pso[qch * HL + jt][:],
                                wo_sb[
                                    :,
                                    dc * DSH + jt * 128 : dc * DSH
                                    + (jt + 1) * 128,
                                ],
                                at_t[:, qch * 512 : (qch + 1) * 512],
                                start=(dc == 0),
                                stop=(dc == KT - 1),
                            )
                for qch in range(2):
                    for jt in range(HL):
                        oev = p3_ev.tile([128, 512], BF16, name="oev")
                        nc.vector.tensor_copy(oev[:], pso[qch * HL + jt][:])
                        nc.scalar.dma_start(
                            outT.ap()[
                                jt * 128 : (jt + 1) * 128,
                                (qcp * 2 + qch) * 512 : (qcp * 2 + qch) * 512
                                + 512,
                            ],
                            oev[:],
                        )

    nc.compile()
    return nc


def _install_trace_hooks():
    """Install the NTFF profile hook (missing antenv.axon_hooks stub) and
    neutralize the artifact upload so trace=True works in this container."""
    import sys
    import types

    from concourse import bass_utils as _bu

    _bu.upload_artifacts = lambda tmpdir: f"file://{tmpdir}"
    if "antenv.axon_hooks" in sys.modules:
        return
    import antenv

    mod = types.ModuleType("antenv.axon_hooks")
    _h = [None]
    mod.set_axon_ntff_profile_hook = lambda hk: _h.__setitem__(0, hk)
    mod.get_axon_ntff_profile_hook = lambda: _h[0]
    sys.modules["antenv.axon_hooks"] = mod
    antenv.axon_hooks = mod
    from trn_agent_boot.trn_boot import _ntff_profile_via_ctypes

    mod.set_axon_ntff_profile_hook(
        _ntff_profile_via_ctypes("/opt/axon/libaxon_pjrt.so")
    )


_CACHE = {}


def _get_program(cls_grid, n_patterns):
    key = (tuple(map(tuple, cls_grid)), n_patterns)
    if key not in _CACHE:
        _CACHE[key] = _build(cls_grid, n_patterns)
    return _CACHE[key]


def _classify_mask(maskT_np):
    """Classify each [128k, 512q] block of the transposed mask; mixed blocks
    get an index into a deduplicated list of additive patterns."""
    grid = []
    patterns = []
    for kt in range(ST):
        row = []
        for qc in range(SC):
            blk = maskT_np[kt * 128 : (kt + 1) * 128, qc * 512 : (qc + 1) * 512]
            if np.all(blk < -1e4):
                row.append(B_SKIP)
            elif np.all(blk == 0.0):
                row.append(B_ZERO)
            else:
                for i, p in enumerate(patterns):
                    if np.array_equal(p, blk):
                        row.append(i)
                        break
                else:
                    patterns.append(blk.copy())
                    row.append(len(patterns) - 1)
        grid.append(row)
    return grid, patterns


_ONES = np.ones((128, 1), dtype=ml_dtypes.bfloat16)

# within-head permutation: even head_dim indices first, then odd
_PERM = np.empty(DSH, dtype=np.int64)
for _hl in range(HL):
    for _j in range(64):
        _PERM[_hl * 128 + _j] = _hl * 128 + 2 * _j
        _PERM[_hl * 128 + 64 + _j] = _hl * 128 + 2 * _j + 1


def kernel(x, start_pos, freqs, mask, wq, wk, wv, wo):
    x = np.asarray(x, dtype=np.float32)
    freqs = np.asarray(freqs, dtype=np.float32)
    mask = np.asarray(mask, dtype=np.float32)
    wq = np.asarray(wq, dtype=np.float32)
    wk = np.asarray(wk, dtype=np.float32)
    wv = np.asarray(wv, dtype=np.float32)
    wo = np.asarray(wo, dtype=np.float32)

    bf = ml_dtypes.bfloat16
    xs = x.reshape(S, D)
    xT = np.ascontiguousarray(xs.T).astype(bf)

    # rotary multipliers in the permuted-hd layout: rows 0-63 = cos-sin,
    # rows 64-127 = cos+sin; gq additionally folds the 1/sqrt(HD) scale.
    cos = freqs[:, :, 0].T  # (64, S)
    sin = freqs[:, :, 1].T
    gk_np = np.ascontiguousarray(
        np.concatenate([cos - sin, cos + sin], axis=0), dtype=np.float32
    )
    gq_np = np.ascontiguousarray(gk_np / math.sqrt(HD), dtype=np.float32)

    maskT_np = np.ascontiguousarray(mask.reshape(S, S).T)
    cls_grid, patterns = _classify_mask(maskT_np)
    n_pat = len(patterns)
    # each pattern duplicated side by side for the head-pair [128,1024] add
    maskp_np = np.zeros((128, max(n_pat, 1) * 1024), dtype=bf)
    for i, p in enumerate(patterns):
        pb = p.astype(bf)
        maskp_np[:, i * 1024 : i * 1024 + 512] = pb
        maskp_np[:, i * 1024 + 512 : (i + 1) * 1024] = pb

    nc = _get_program(cls_grid, n_pat)

    in_maps = []
    for c in range(N_CORES):
        rows = slice(c * DSH, (c + 1) * DSH)
        wq_c = wq[rows][_PERM]  # permute within-head rows (even hd, odd hd)
        wk_c = wk[rows][_PERM]
        in_maps.append(
            {
                "xT": xT,
                "wqT": np.ascontiguousarray(wq_c.T).astype(bf),
                "wkT": np.ascontiguousarray(wk_c.T).astype(bf),
                "wvT": np.ascontiguousarray(wv[rows].T).astype(bf),
                "woT": np.ascontiguousarray(wo[rows].T).astype(bf),
                "gq": gq_np,
                "gk": gk_np,
                "ones_in": _ONES,
                "maskp": maskp_np,
            }
        )

    trace = os.environ.get("ATTN_TRACE") == "1"
    if trace:
        try:
            _install_trace_hooks()
        except Exception:
            pass

    res = run_bass_kernel_spmd(
        nc,
        in_maps,
        list(range(N_CORES)),
        trace=trace,
        trace_cores=[0] if trace else None,
    )
    if trace:
        kernel.last_exec_time_ns = res.exec_time_ns
        kernel.last_results = res

    out = np.empty((S, D), dtype=np.float32)
    for c in range(N_CORES):
        out[:, c * DSH : (c + 1) * DSH] = res.results[c]["outT"].T.astype(
            np.float32
        )
    return out[None]
